# revision 28
# baseline (speedup 1.0000x reference)
"""Multi-head causal attention (B=2,S=2048,D=1024,H=16,RoPE) on 8 TRN2 NeuronCores.

Sharding: core c handles batch b=c//4, head-group g=c%4 (4 heads each).
Wq/Wk/Wv column-sharded per head group, Wo row-sharded; the all-reduce over
head groups is realized as a host-side partial sum at gather time.

v3: all matmul operands bf16 (fp32 PSUM). Score pair for both heads of an
m-tile lands in one 2-bank PSUM tile so a single ACTIVATE does exp for both
(the ~172-cycle ACT overhead amortizes). The PE is kept continuously busy --
QKV-projection chains for the NEXT query block and output-projection chains
for the PREVIOUS one are interleaved into the attention kt loop as fillers;
PE idle gaps would re-engage the HAM clock throttle (PE drops 2.4->1.2 GHz
after ~3.4us windows containing idle), which is what made fp32->bf16 alone
a wash. vext pads each head's V+ones chunk to 66 bf16 elements so the ACT
copy (V cols) and the vones DMA (ones col) never share a 4-byte SBUF word
(observed RMW corruption at odd heads with stride 65).

Per-core kernel:
  proj(sb): psq [d,s] chains -> ACT copy (+bq bias) -> RoPE (DVE, bf16 2x);
            V natural [s,d] with ones column riding the AV matmul.
  attn(qb,m): scoresT[k,q] for hp=0,1 -> fused exp -> diag mask (GPSIMD) ->
            AV into puv [65,2,512]; vec evacuated to SBUF early (uv pool is
            single-buffered), denominators reciprocal'd via a DRAM
            spread/broadcast round trip; out-proj per qb as fillers.
"""
import numpy as np
import ml_dtypes
from contextlib import ExitStack

import concourse.bass as bass
import concourse.tile as tile
from concourse import mybir
from concourse.bass_utils import run_bass_kernel_spmd

B, S, D, H, HD = 2, 2048, 1024, 16, 64
HPC = 4            # heads per core
DC = HPC * HD      # 256 features per core
NDT = D // 128     # 8 input-dim tiles
NST = S // 128     # 16 sequence/key tiles
NQB = S // 512     # 4 query blocks
MT = DC // 128     # 2 feature m-tiles for Q/K/vec
EV = 66            # padded per-head V chunk (64 V + 1 ones + 1 pad)

F32 = mybir.dt.float32
BF16 = mybir.dt.bfloat16
AF = mybir.ActivationFunctionType
BF = ml_dtypes.bfloat16

_nop_ctr = [0]


def fix_engine_waits(nc, max_waits=1):
    """This walrus build rejects any engine instruction with >1 sync wait
    (single wait slot per instruction struct). Move excess waits onto
    same-engine NoOps inserted just before, one wait per NoOp. InstISA is
    skipped (fixed-length encoding)."""
    moved = 0
    for f in nc.m.functions:
        for b in f.blocks:
            insts = b.instructions
            i = 0
            while i < len(insts):
                inst = insts[i]
                if inst.opcode != "ISA" and inst.sync_info is not None:
                    si = inst.sync_info
                    waits = list(si.on_wait)
                    if len(waits) > max_waits:
                        keep = waits[-max_waits:]
                        for w in waits[:-max_waits]:
                            _nop_ctr[0] += 1
                            moved += 1
                            nop = mybir.InstNoOp(
                                name=f"I-waitnop-{_nop_ctr[0]}", ins=[], outs=[]
                            )
                            nop.engine = inst.engine
                            nop.sync_info = mybir.SyncInfo(on_wait=[w], on_update=[])
                            insts.insert(i, nop)
                            i += 1
                        si.on_wait = keep
                        inst.sync_info = si
                i += 1
    return moved


def _attention_body(ctx: ExitStack, tc, inp, out_ap):
    nc = tc.nc

    persist = ctx.enter_context(tc.tile_pool(name="persist", bufs=1))
    exp_p = ctx.enter_context(tc.tile_pool(name="expp", bufs=4))
    rrow_p = ctx.enter_context(tc.tile_pool(name="rrow", bufs=2))
    rec_p = ctx.enter_context(tc.tile_pool(name="recp", bufs=2))
    vu_p = ctx.enter_context(tc.tile_pool(name="vup", bufs=2))
    vtmp_p = ctx.enter_context(tc.tile_pool(name="vtmp", bufs=2))
    tout_p = ctx.enter_context(tc.tile_pool(name="toutp", bufs=2))
    ps_pool = ctx.enter_context(tc.tile_pool(name="ps", bufs=2, space="PSUM"))
    wpool = ctx.enter_context(tc.tile_pool(name="wpool", bufs=1))
    xtp = ctx.enter_context(tc.tile_pool(name="xtp", bufs=2))
    qtmp_p = ctx.enter_context(tc.tile_pool(name="qtmp", bufs=3))
    tb2_p = ctx.enter_context(tc.tile_pool(name="tb2", bufs=2))

    # ---- persistent tensors ----
    qrt = persist.tile([128, MT, S], BF16)      # rotated Q^T  (d-major)
    krt = persist.tile([128, MT, S], BF16)      # rotated K^T
    vext = persist.tile([128, NST, HPC * EV], BF16)  # V tiles + ones col per head
    vecT = persist.tile([128, MT, S], BF16)     # normalized attention output^T
    cos_sb = persist.tile([128, S], BF16)
    sin_sb = persist.tile([128, S], BF16)
    wo_sb = persist.tile([128, MT, D], BF16)
    mdiag_sb = persist.tile([128, 128], BF16)   # binary causal mask, diag block^T
    bq_sb = persist.tile([128, MT], F32)
    bv_sb = persist.tile([1, DC], BF16)
    ones_sb = persist.tile([1, 128], BF16)
    pshift_sb = persist.tile([128, 128], BF16)

    # weights first (per d-tile so the first matmuls start early), then consts
    wq_sb = wpool.tile([128, NDT, DC], BF16)
    wk_sb = wpool.tile([128, NDT, DC], BF16)
    wv_sb = wpool.tile([128, NDT, DC], BF16)
    xT_view = inp["xT"].rearrange("(dt p) s -> p dt s", p=128)
    wq_view = inp["wqT"].rearrange("(dt p) o -> p dt o", p=128)
    xt0 = xtp.tile([128, NDT, 512], BF16, tag="xt", name="xt0")
    # first chain's inputs land per-d-tile so its matmuls start early
    # first chain's inputs land per-d-tile so its matmuls start early
    for dt in range(NDT):
        nc.sync.dma_start(wq_sb[:, dt, :], wq_view[:, dt, :])
        nc.sync.dma_start(xt0[:, dt, :], xT_view[:, dt, 0:512])
    for w_sb, nm in ((wk_sb, "wkT"), (wv_sb, "wvT")):
        nc.sync.dma_start(
            w_sb[:, :, :], inp[nm].rearrange("(dt p) o -> p dt o", p=128)
        )
    nc.sync.dma_start(cos_sb[:, :], inp["cosT"])
    nc.sync.dma_start(sin_sb[:, :], inp["sinT"])
    nc.sync.dma_start(mdiag_sb[:, :], inp["mdiagT"])
    nc.sync.dma_start(bq_sb[:, :], inp["bqc"])
    nc.sync.dma_start(bv_sb[:, :], inp["bvr"])
    nc.sync.dma_start(ones_sb[:, :], inp["ones"])
    nc.sync.dma_start(pshift_sb[:, :], inp["pshift"])
    # ones columns of vext (col 64 of each head slot, every k-tile)
    vones_dst = vext[:, :, :].rearrange("p st (h e) -> p st h e", e=EV)[:, :, :, 64:65]
    nc.sync.dma_start(vones_dst, inp["vones"].rearrange("p (st h e) -> p st h e", st=NST, h=HPC))
    nc.sync.dma_start(
        wo_sb[:, :, :], inp["woT"].rearrange("(mt p) o -> p mt o", p=128)
    )

    # ---- Phase A chunks ----
    # qk_stage1: projection chain -> qt (SBUF). qk_stage2 (pipelined one chunk
    # behind so the PE never waits on the qt copy): rotate_half as a PE
    # permutation matmul (sign baked into pshift), then RoPE combine on DVE.
    def qk_stage1(m, sb, xt, which):
        ssl = slice(sb * 512, (sb + 1) * 512)
        w_sb, dst, is_q = (wq_sb, qrt, True) if which == 0 else (wk_sb, krt, False)
        tag_q = "q" if is_q else "k"
        psq = ps_pool.tile([128, 512], F32, tag="w", name=f"psq{tag_q}_{m}_{sb}")
        for dt in range(NDT):
            nc.tensor.matmul(
                psq[:, :],
                w_sb[:, dt, m * 128 : (m + 1) * 128],
                xt[:, dt, :],
                start=(dt == 0),
                stop=(dt == NDT - 1),
            )
        qt = qtmp_p.tile([128, 512], BF16, tag="qt", name=f"qt{tag_q}_{m}_{sb}")
        if is_q:
            nc.scalar.activation(
                qt[:, :], psq[:, :], AF.Identity, bias=bq_sb[:, m : m + 1]
            )
        else:
            nc.scalar.copy(qt[:, :], psq[:, :])
        return qt, dst[:, m, ssl], ssl

    def qk_stage2(state):
        qt, dsl, ssl = state
        psh = ps_pool.tile([128, 512], F32, tag="w", name="psh")
        nc.tensor.matmul(psh[:, :], pshift_sb[:, :], qt[:, :], start=True, stop=True)
        tb2 = tb2_p.tile([128, 512], BF16, tag="tb2", name="tb2")
        nc.vector.tensor_mul(dsl, qt[:, :], cos_sb[:, ssl])
        nc.vector.tensor_mul(tb2[:, :], psh[:, :], sin_sb[:, ssl])
        nc.vector.tensor_add(dsl, dsl, tb2[:, :])

    def v_proj(st, xt):
        psv = ps_pool.tile([128, 512], F32, tag="w", name=f"psv_{st}")[:, 0:256]
        for dt in range(NDT):
            nc.tensor.matmul(
                psv[:, :],
                xt[:, dt, (st % 4) * 128 : (st % 4 + 1) * 128],
                wv_sb[:, dt, :],
                start=(dt == 0),
                stop=False,
            )
        nc.tensor.matmul(
            psv[:, :], ones_sb[0:1, :], bv_sb[0:1, :], start=False, stop=True
        )
        vdst = vext[:, st, :].rearrange("p (h e) -> p h e", e=EV)[:, :, 0:64]
        nc.scalar.copy(vdst, psv[:, :].rearrange("p (h e) -> p h e", e=64))

    def proj_chunks(sb, xt):
        # qk stage2 trails stage1 by one chunk so the rotate matmul's qt
        # input is already copied when it reaches the head of the PE queue.
        st = {}

        def qk(i, m, which):
            def run():
                prev = st.pop("p", None)
                st["p"] = qk_stage1(m, sb, xt, which)
                if prev is not None:
                    qk_stage2(prev)
            return run

        def vp(st_i):
            return lambda: v_proj(st_i, xt)

        def last():
            v_proj(4 * sb + 3, xt)
            qk_stage2(st.pop("p"))

        return [
            qk(0, 0, 0), qk(1, 1, 0), vp(4 * sb), vp(4 * sb + 1),
            qk(2, 0, 1), qk(3, 1, 1), vp(4 * sb + 2), last,
        ]

    def normalize_pair(m, qb, vu, srow):
        """Reciprocal of both heads' denominators (SBUF spread, one DRAM hop
        for the partition broadcast), then the two normalize multiplies."""
        ridx = m * NQB + qb
        # spread the 1024 sums over 32 partitions so reciprocal runs parallel
        scol = rec_p.tile([32, 32], F32, tag="sc", name=f"scol_{m}_{qb}")
        nc.scalar.dma_start(scol[:, :], srow[0:1, :, :].rearrange("one a b -> one (a b)"))
        rcol = rec_p.tile([32, 32], BF16, tag="rb", name=f"rcol_{m}_{qb}")
        with nc.allow_low_precision(reason="bf16 softmax denominators; tol 2e-2"):
            nc.vector.reciprocal(rcol[:, :], scol[:, :])
        rspread = inp["recr"][ridx : ridx + 1, :].rearrange(
            "one (p j) -> (one p) j", j=32
        )
        nc.scalar.dma_start(rspread, rcol[:, :])
        rec = rec_p.tile([64, 2, 512], BF16, tag="rc", name=f"rec_{m}_{qb}")
        nc.scalar.dma_start(
            rec[:, :, :],
            inp["recr"][ridx : ridx + 1, :].to_broadcast([64, 1024]).rearrange(
                "p (a b) -> p a b", a=2
            ),
        )
        qsl = slice(qb * 512, (qb + 1) * 512)
        nc.vector.tensor_mul(vecT[0:64, m, qsl], vu[:, 0, :], rec[:, 0, :])
        vt = vtmp_p.tile([64, 512], BF16, tag="vt", name=f"vt_{m}_{qb}")
        nc.vector.tensor_mul(vt[:, :], vu[:, 1, :], rec[:, 1, :])
        nc.scalar.dma_start(vecT[64:128, m, qsl], vt[:, :])

    xts = [xt0]
    for sb in range(1, NQB):
        xtn = xtp.tile([128, NDT, 512], BF16, tag="xt", name=f"xt0_{sb}")
        for dt in range(NDT):
            nc.sync.dma_start(
                xtn[:, dt, :], xT_view[:, dt, sb * 512 : (sb + 1) * 512]
            )
        xts.append(xtn)

    def attn_qb(m, qb, fillers, pace):
        puv = ps_pool.tile([65, 2, 512], F32, tag="uv", bufs=1, name=f"puv_m{m}q{qb}")
        pending = None
        for kt in range(4 * qb + 5):
            if kt <= 4 * qb + 3:
                qb0 = kt // 4
                c0 = (kt % 4) * 128 if qb == qb0 else 0
                pse = ps_pool.tile([128, 2, 512], F32, tag="ps", name=f"pse{m}_{qb}_{kt}")
                for hp in range(2):
                    pb = hp * 64
                    nc.tensor.matmul(
                        pse[:, hp, c0:512],
                        krt[pb : pb + 64, m, kt * 128 : (kt + 1) * 128],
                        qrt[pb : pb + 64, m, qb * 512 + c0 : (qb + 1) * 512],
                        start=True,
                        stop=True,
                    )
                et = exp_p.tile([128, 2, 512], BF16, tag="e", name=f"et{m}_{qb}_{kt}")
                nc.scalar.activation(
                    et[:, :, c0:512], pse[:, :, c0:512], AF.Exp, scale=0.125
                )
                if qb == qb0:
                    for hp in range(2):
                        nc.gpsimd.tensor_mul(
                            et[:, hp, c0 : c0 + 128],
                            et[:, hp, c0 : c0 + 128],
                            mdiag_sb[:, :],
                        )
            else:
                et, c0 = None, None
            if pending is not None:
                pkt, pc0, pet = pending
                for hp in range(2):
                    h = 2 * m + hp
                    nc.tensor.matmul(
                        puv[:, hp, pc0:512],
                        vext[:, pkt, h * EV : h * EV + 65],
                        pet[:, hp, pc0:512],
                        start=(pkt == 0),
                        stop=(pkt == qb * 4 + 3),
                        skip_group_check=True,
                    )
            pending = (kt, c0, et) if et is not None else None
            if fillers and (kt % pace) == pace - 1:
                fillers.pop(0)()
        # evacuate puv quickly; the denominator row first (it heads the
        # critical reciprocal chain), then the vec cast (uv bufs=1)
        srow = rrow_p.tile([1, 2, 512], F32, tag="rr", name=f"srow_{m}_{qb}")
        nc.vector.tensor_copy(srow[:, :, :], puv[64:65, :, :])
        vu = vu_p.tile([64, 2, 512], BF16, tag="vu", name=f"vu_{m}_{qb}")
        nc.vector.tensor_copy(vu[:, :, :], puv[0:64, :, :])
        normalize_pair(m, qb, vu, srow)
        while fillers:
            fillers.pop(0)()

    to_tiles = {}

    def outproj_chunk(qt_i, oc):
        qsl = slice(qt_i * 128, (qt_i + 1) * 128)
        osl = slice(oc * 512, (oc + 1) * 512)
        pso = ps_pool.tile([128, 512], F32, tag="w", name="pso")
        for mt in range(MT):
            nc.tensor.matmul(
                pso[:, :],
                vecT[:, mt, qsl],
                wo_sb[:, mt, osl],
                start=(mt == 0),
                stop=(mt == MT - 1),
            )
        to = tout_p.tile([128, 512], BF16, tag="to", name=f"to_{qt_i}_{oc}")
        if (qt_i + oc) % 2 == 0:
            nc.scalar.copy(to[:, :], pso[:, :])
        else:
            nc.vector.tensor_copy(to[:, :], pso[:, :])
        nc.sync.dma_start(out_ap[qsl, osl], to[:, :])

    def outproj_chunks(qb):
        return [
            (lambda qt_i=qt_i, oc=oc: outproj_chunk(qt_i, oc))
            for qt_i in range(4 * qb, 4 * qb + 4)
            for oc in range(2)
        ]

    # ---- main pipeline ----
    for c in proj_chunks(0, xts[0]):
        c()
    for qb in range(NQB):
        fillers = []
        if qb + 1 < NQB:
            fillers += proj_chunks(qb + 1, xts[qb + 1])
        if qb >= 1:
            fillers += outproj_chunks(qb - 1)
        points = 2 * (4 * qb + 5)
        pace = max(1, points // max(1, len(fillers)))
        half = (len(fillers) + 1) // 2
        attn_qb(0, qb, fillers[:half], pace)
        # for the final block, hold the fillers past the kt loop so they run
        # during the last normalize chain (keeps HAM warm for outproj)
        attn_qb(1, qb, fillers[half:], 10**6 if qb == NQB - 1 else pace)
    for c in outproj_chunks(NQB - 1):
        c()
    ctx._dbg = {"qrt": qrt, "krt": krt, "vext": vext, "vecT": vecT}


def build_bass(fix_waits=True, debug_dump=False):
    nc = bass.Bass("TRN2", debug=False)
    inp = {}

    def din(name, shape, dtype=BF16):
        inp[name] = nc.dram_tensor(name, list(shape), dtype, kind="ExternalInput").ap()

    din("xT", (D, S))
    din("wqT", (D, DC))
    din("wkT", (D, DC))
    din("wvT", (D, DC))
    din("bqc", (128, MT), F32)
    din("bvr", (1, DC))
    din("cosT", (128, S))
    din("sinT", (128, S))
    din("mdiagT", (128, 128))
    din("woT", (DC, D))
    din("ones", (1, 128))
    din("pshift", (128, 128))
    din("vones", (128, NST * HPC))
    inp["recr"] = nc.dram_tensor("recr", [MT * NQB, 1024], BF16, kind="Internal").ap()
    out_ap = nc.dram_tensor("out", [S, D], BF16, kind="ExternalOutput").ap()

    if debug_dump:
        for nm, shape in (("qrt_d", [128, MT * S]), ("krt_d", [128, MT * S]),
                          ("vext_d", [128, NST * HPC * EV]),
                          ("vecT_d", [128, MT * S])):
            inp[nm] = nc.dram_tensor(nm, shape, BF16, kind="ExternalOutput").ap()

    with tile.TileContext(nc) as tc:
        with ExitStack() as ctx:
            _attention_body(ctx, tc, inp, out_ap)
            if debug_dump:
                nc.sync.dma_start(inp["qrt_d"], ctx._dbg["qrt"].rearrange("p a b -> p (a b)"))
                nc.sync.dma_start(inp["krt_d"], ctx._dbg["krt"].rearrange("p a b -> p (a b)"))
                nc.sync.dma_start(inp["vext_d"], ctx._dbg["vext"].rearrange("p a b -> p (a b)"))
                nc.sync.dma_start(inp["vecT_d"], ctx._dbg["vecT"].rearrange("p a b -> p (a b)"))
    if fix_waits:
        fix_engine_waits(nc)
    return nc


# ---- host-side sharding / prep ----


def make_core_inputs(x, mask, cos, sin, wq, bq, wk, wv, bv, wo):
    """Returns list of 8 input dicts (core c = batch c//4, head-group c%4)."""
    x = np.ascontiguousarray(x, dtype=np.float32)
    p = np.arange(128)
    pf = p % 64
    cosT = np.ascontiguousarray(cos.T[pf, :].astype(BF))                 # [128, S]
    sinT = np.ascontiguousarray(sin.T[pf, :].astype(BF))
    # rotate_half as a permutation matmul: psh[p,s] = sgn(p) * qt[p^32, s]
    # pshift is the stationary lhsT: pshift[q, p] = sgn(p) * (q == p ^ 32)
    sgn = np.where(pf < 32, -1.0, 1.0).astype(np.float32)
    pshift = np.zeros((128, 128), dtype=np.float32)
    pshift[p ^ 32, p] = sgn
    pshift = np.ascontiguousarray(pshift.astype(BF))
    mdiagT = np.ascontiguousarray(
        (mask[0:128, 0:128].T == 0).astype(BF)
    )
    ones = np.ones((1, 128), dtype=BF)
    vones = np.ones((128, NST * HPC), dtype=BF)

    in_maps = []
    for c in range(8):
        b, g = c // 4, c % 4
        rows = np.arange(g * DC, (g + 1) * DC)
        vrows = rows
        bqc = np.ascontiguousarray(bq[rows].reshape(MT, 128).T, dtype=np.float32)
        in_maps.append({
            "xT": np.ascontiguousarray(x[b].T.astype(BF)),
            "wqT": np.ascontiguousarray(wq[rows].T.astype(BF)),
            "wkT": np.ascontiguousarray(wk[rows].T.astype(BF)),
            "wvT": np.ascontiguousarray(wv[vrows].T.astype(BF)),
            "bqc": bqc,
            "bvr": np.ascontiguousarray(bv[vrows][None, :].astype(BF)),
            "cosT": cosT,
            "sinT": sinT,
            "mdiagT": mdiagT,
            "woT": np.ascontiguousarray(wo[:, vrows].T.astype(BF)),
            "ones": ones,
            "pshift": pshift,
            "vones": vones,
        })
    return in_maps


_NC_CACHE = []


def kernel(x, mask, cos, sin, wq, bq, wk, wv, bv, wo, bo):
    x = np.asarray(x, dtype=np.float32)
    in_maps = make_core_inputs(
        x, np.asarray(mask), np.asarray(cos), np.asarray(sin),
        np.asarray(wq), np.asarray(bq), np.asarray(wk), np.asarray(wv),
        np.asarray(bv), np.asarray(wo),
    )
    if not _NC_CACHE:
        _NC_CACHE.append(build_bass())
    nc = _NC_CACHE[0]
    res = run_bass_kernel_spmd(nc, in_maps, core_ids=list(range(8)))
    out = np.zeros((B, S, D), dtype=np.float32)
    for c in range(8):
        out[c // 4] += res.results[c]["out"].astype(np.float32)
    out += np.asarray(bo, dtype=np.float32)[None, None, :]
    return out


# revision 29
# speedup vs baseline: 1.0653x; 1.0653x over previous
"""Multi-head causal attention (B=2,S=2048,D=1024,H=16,RoPE) on 8 TRN2 NeuronCores.

Sharding: core c handles batch b=c//4, head-group g=c%4 (4 heads each).
Wq/Wk/Wv column-sharded per head group, Wo row-sharded; the all-reduce over
head groups is realized as a host-side partial sum at gather time.

v3: all matmul operands bf16 (fp32 PSUM). Score pair for both heads of an
m-tile lands in one 2-bank PSUM tile so a single ACTIVATE does exp for both
(the ~172-cycle ACT overhead amortizes). The PE is kept continuously busy --
QKV-projection chains for the NEXT query block and output-projection chains
for the PREVIOUS one are interleaved into the attention kt loop as fillers;
PE idle gaps would re-engage the HAM clock throttle (PE drops 2.4->1.2 GHz
after ~3.4us windows containing idle), which is what made fp32->bf16 alone
a wash. vext pads each head's V+ones chunk to 66 bf16 elements so the ACT
copy (V cols) and the vones DMA (ones col) never share a 4-byte SBUF word
(observed RMW corruption at odd heads with stride 65).

Per-core kernel:
  proj(sb): psq [d,s] chains -> ACT copy (+bq bias) -> RoPE (DVE, bf16 2x);
            V natural [s,d] with ones column riding the AV matmul.
  attn(qb,m): scoresT[k,q] for hp=0,1 -> fused exp -> diag mask (GPSIMD) ->
            AV into puv [65,2,512]; vec evacuated to SBUF early (uv pool is
            single-buffered), denominators reciprocal'd via a DRAM
            spread/broadcast round trip; out-proj per qb as fillers.
"""
import numpy as np
import ml_dtypes
from contextlib import ExitStack

import concourse.bass as bass
import concourse.tile as tile
from concourse import mybir
from concourse.bass_utils import run_bass_kernel_spmd

B, S, D, H, HD = 2, 2048, 1024, 16, 64
HPC = 4            # heads per core
DC = HPC * HD      # 256 features per core
NDT = D // 128     # 8 input-dim tiles
NST = S // 128     # 16 sequence/key tiles
NQB = S // 512     # 4 query blocks
MT = DC // 128     # 2 feature m-tiles for Q/K/vec
EV = 66            # padded per-head V chunk (64 V + 1 ones + 1 pad)

F32 = mybir.dt.float32
BF16 = mybir.dt.bfloat16
AF = mybir.ActivationFunctionType
BF = ml_dtypes.bfloat16

_nop_ctr = [0]


def fix_engine_waits(nc, max_waits=1):
    """This walrus build rejects any engine instruction with >1 sync wait
    (single wait slot per instruction struct). Move excess waits onto
    same-engine NoOps inserted just before, one wait per NoOp. InstISA is
    skipped (fixed-length encoding)."""
    moved = 0
    for f in nc.m.functions:
        for b in f.blocks:
            insts = b.instructions
            i = 0
            while i < len(insts):
                inst = insts[i]
                if inst.opcode != "ISA" and inst.sync_info is not None:
                    si = inst.sync_info
                    waits = list(si.on_wait)
                    if len(waits) > max_waits:
                        keep = waits[-max_waits:]
                        for w in waits[:-max_waits]:
                            _nop_ctr[0] += 1
                            moved += 1
                            nop = mybir.InstNoOp(
                                name=f"I-waitnop-{_nop_ctr[0]}", ins=[], outs=[]
                            )
                            nop.engine = inst.engine
                            nop.sync_info = mybir.SyncInfo(on_wait=[w], on_update=[])
                            insts.insert(i, nop)
                            i += 1
                        si.on_wait = keep
                        inst.sync_info = si
                i += 1
    return moved


def _attention_body(ctx: ExitStack, tc, inp, out_ap):
    nc = tc.nc

    persist = ctx.enter_context(tc.tile_pool(name="persist", bufs=1))
    exp_p = ctx.enter_context(tc.tile_pool(name="expp", bufs=4))
    rrow_p = ctx.enter_context(tc.tile_pool(name="rrow", bufs=2))
    rec_p = ctx.enter_context(tc.tile_pool(name="recp", bufs=2))
    vu_p = ctx.enter_context(tc.tile_pool(name="vup", bufs=2))
    vtmp_p = ctx.enter_context(tc.tile_pool(name="vtmp", bufs=2))
    tout_p = ctx.enter_context(tc.tile_pool(name="toutp", bufs=2))
    ps_pool = ctx.enter_context(tc.tile_pool(name="ps", bufs=2, space="PSUM"))
    wpool = ctx.enter_context(tc.tile_pool(name="wpool", bufs=1))
    xtp = ctx.enter_context(tc.tile_pool(name="xtp", bufs=2))
    qtmp_p = ctx.enter_context(tc.tile_pool(name="qtmp", bufs=3))
    tb2_p = ctx.enter_context(tc.tile_pool(name="tb2", bufs=2))

    # ---- persistent tensors ----
    qrt = persist.tile([128, MT, S], BF16)      # rotated Q^T  (d-major)
    krt = persist.tile([128, MT, S], BF16)      # rotated K^T
    vext = persist.tile([128, NST, HPC * EV], BF16)  # V tiles + ones col per head
    vecT = persist.tile([128, MT, S], BF16)     # normalized attention output^T
    cos_sb = persist.tile([128, S], BF16)
    sin_sb = persist.tile([128, S], BF16)
    wo_sb = persist.tile([128, MT, D], BF16)
    mdiag_sb = persist.tile([128, 128], BF16)   # binary causal mask, diag block^T
    bq_sb = persist.tile([128, MT], F32)
    bv_sb = persist.tile([1, DC], BF16)
    ones_sb = persist.tile([1, 128], BF16)
    pshift_sb = persist.tile([128, 128], BF16)

    # weights first (per d-tile so the first matmuls start early), then consts
    wq_sb = wpool.tile([128, NDT, DC], BF16)
    wk_sb = wpool.tile([128, NDT, DC], BF16)
    wv_sb = wpool.tile([128, NDT, DC], BF16)
    xT_view = inp["xT"].rearrange("(dt p) s -> p dt s", p=128)
    wq_view = inp["wqT"].rearrange("(dt p) o -> p dt o", p=128)
    xt0 = xtp.tile([128, NDT, 512], BF16, tag="xt", name="xt0")
    # first chain's inputs land per-d-tile so its matmuls start early
    # first chain's inputs land per-d-tile so its matmuls start early
    for dt in range(NDT):
        nc.sync.dma_start(wq_sb[:, dt, :], wq_view[:, dt, :])
        nc.sync.dma_start(xt0[:, dt, :], xT_view[:, dt, 0:512])
    for w_sb, nm in ((wk_sb, "wkT"), (wv_sb, "wvT")):
        nc.sync.dma_start(
            w_sb[:, :, :], inp[nm].rearrange("(dt p) o -> p dt o", p=128)
        )
    nc.sync.dma_start(cos_sb[:, :], inp["cosT"])
    nc.sync.dma_start(sin_sb[:, :], inp["sinT"])
    nc.sync.dma_start(mdiag_sb[:, :], inp["mdiagT"])
    nc.sync.dma_start(bq_sb[:, :], inp["bqc"])
    nc.sync.dma_start(bv_sb[:, :], inp["bvr"])
    nc.sync.dma_start(ones_sb[:, :], inp["ones"])
    nc.sync.dma_start(pshift_sb[:, :], inp["pshift"])
    # ones columns of vext (col 64 of each head slot, every k-tile)
    vones_dst = vext[:, :, :].rearrange("p st (h e) -> p st h e", e=EV)[:, :, :, 64:65]
    nc.sync.dma_start(vones_dst, inp["vones"].rearrange("p (st h e) -> p st h e", st=NST, h=HPC))
    nc.sync.dma_start(
        wo_sb[:, :, :], inp["woT"].rearrange("(mt p) o -> p mt o", p=128)
    )

    # ---- Phase A chunks ----
    # qk_stage1: projection chain -> qt (SBUF). qk_stage2 (pipelined one chunk
    # behind so the PE never waits on the qt copy): rotate_half as a PE
    # permutation matmul (sign baked into pshift), then RoPE combine on DVE.
    def qk_stage1(m, sb, xt, which):
        ssl = slice(sb * 512, (sb + 1) * 512)
        w_sb, dst, is_q = (wq_sb, qrt, True) if which == 0 else (wk_sb, krt, False)
        tag_q = "q" if is_q else "k"
        psq = ps_pool.tile([128, 512], F32, tag="w", name=f"psq{tag_q}_{m}_{sb}")
        for dt in range(NDT):
            nc.tensor.matmul(
                psq[:, :],
                w_sb[:, dt, m * 128 : (m + 1) * 128],
                xt[:, dt, :],
                start=(dt == 0),
                stop=(dt == NDT - 1),
            )
        qt = qtmp_p.tile([128, 512], BF16, tag="qt", name=f"qt{tag_q}_{m}_{sb}")
        if is_q:
            nc.scalar.activation(
                qt[:, :], psq[:, :], AF.Identity, bias=bq_sb[:, m : m + 1]
            )
        else:
            nc.scalar.copy(qt[:, :], psq[:, :])
        return qt, dst[:, m, ssl], ssl

    def qk_stage2(state):
        qt, dsl, ssl = state
        psh = ps_pool.tile([128, 512], F32, tag="w", name="psh")
        nc.tensor.matmul(psh[:, :], pshift_sb[:, :], qt[:, :], start=True, stop=True)
        tb2 = tb2_p.tile([128, 512], BF16, tag="tb2", name="tb2")
        nc.vector.tensor_mul(dsl, qt[:, :], cos_sb[:, ssl])
        nc.vector.tensor_mul(tb2[:, :], psh[:, :], sin_sb[:, ssl])
        nc.vector.tensor_add(dsl, dsl, tb2[:, :])

    def v_proj(st, xt):
        psv = ps_pool.tile([128, 512], F32, tag="w", name=f"psv_{st}")[:, 0:256]
        for dt in range(NDT):
            nc.tensor.matmul(
                psv[:, :],
                xt[:, dt, (st % 4) * 128 : (st % 4 + 1) * 128],
                wv_sb[:, dt, :],
                start=(dt == 0),
                stop=False,
            )
        nc.tensor.matmul(
            psv[:, :], ones_sb[0:1, :], bv_sb[0:1, :], start=False, stop=True
        )
        vdst = vext[:, st, :].rearrange("p (h e) -> p h e", e=EV)[:, :, 0:64]
        nc.scalar.copy(vdst, psv[:, :].rearrange("p (h e) -> p h e", e=64))

    def proj_chunks(sb, xt):
        # qk stage2 trails stage1 by one chunk so the rotate matmul's qt
        # input is already copied when it reaches the head of the PE queue.
        st = {}

        def qk(i, m, which):
            def run():
                prev = st.pop("p", None)
                st["p"] = qk_stage1(m, sb, xt, which)
                if prev is not None:
                    qk_stage2(prev)
            return run

        def vp(st_i):
            return lambda: v_proj(st_i, xt)

        def last():
            v_proj(4 * sb + 3, xt)
            qk_stage2(st.pop("p"))

        return [
            qk(0, 0, 0), qk(1, 1, 0), vp(4 * sb), vp(4 * sb + 1),
            qk(2, 0, 1), qk(3, 1, 1), vp(4 * sb + 2), last,
        ]

    def normalize_pair(m, qb, vu, srow):
        """Reciprocal of both heads' denominators (SBUF spread, one DRAM hop
        for the partition broadcast), then the two normalize multiplies."""
        ridx = m * NQB + qb
        # spread the 1024 sums over 32 partitions so reciprocal runs parallel
        scol = rec_p.tile([32, 32], F32, tag="sc", name=f"scol_{m}_{qb}")
        nc.sync.dma_start(scol[:, :], srow[0:1, :, :].rearrange("one a b -> one (a b)"))
        rcol = rec_p.tile([32, 32], BF16, tag="rb", name=f"rcol_{m}_{qb}")
        with nc.allow_low_precision(reason="bf16 softmax denominators; tol 2e-2"):
            nc.vector.reciprocal(rcol[:, :], scol[:, :])
        rspread = inp["recr"][ridx : ridx + 1, :].rearrange(
            "one (p j) -> (one p) j", j=32
        )
        nc.sync.dma_start(rspread, rcol[:, :])
        rec = rec_p.tile([64, 2, 512], BF16, tag="rc", name=f"rec_{m}_{qb}")
        nc.sync.dma_start(
            rec[:, :, :],
            inp["recr"][ridx : ridx + 1, :].to_broadcast([64, 1024]).rearrange(
                "p (a b) -> p a b", a=2
            ),
        )
        qsl = slice(qb * 512, (qb + 1) * 512)
        nc.vector.tensor_mul(vecT[0:64, m, qsl], vu[:, 0, :], rec[:, 0, :])
        vt = vtmp_p.tile([64, 512], BF16, tag="vt", name=f"vt_{m}_{qb}")
        nc.vector.tensor_mul(vt[:, :], vu[:, 1, :], rec[:, 1, :])
        nc.sync.dma_start(vecT[64:128, m, qsl], vt[:, :])

    xts = [xt0]
    for sb in range(1, NQB):
        xtn = xtp.tile([128, NDT, 512], BF16, tag="xt", name=f"xt0_{sb}")
        for dt in range(NDT):
            nc.sync.dma_start(
                xtn[:, dt, :], xT_view[:, dt, sb * 512 : (sb + 1) * 512]
            )
        xts.append(xtn)

    def attn_qb(m, qb, fillers, pace):
        puv = ps_pool.tile([65, 2, 512], F32, tag="uv", bufs=1, name=f"puv_m{m}q{qb}")
        pending = None
        for kt in range(4 * qb + 5):
            if kt <= 4 * qb + 3:
                qb0 = kt // 4
                c0 = (kt % 4) * 128 if qb == qb0 else 0
                pse = ps_pool.tile([128, 2, 512], F32, tag="ps", name=f"pse{m}_{qb}_{kt}")
                for hp in range(2):
                    pb = hp * 64
                    nc.tensor.matmul(
                        pse[:, hp, c0:512],
                        krt[pb : pb + 64, m, kt * 128 : (kt + 1) * 128],
                        qrt[pb : pb + 64, m, qb * 512 + c0 : (qb + 1) * 512],
                        start=True,
                        stop=True,
                    )
                et = exp_p.tile([128, 2, 512], BF16, tag="e", name=f"et{m}_{qb}_{kt}")
                nc.scalar.activation(
                    et[:, :, c0:512], pse[:, :, c0:512], AF.Exp, scale=0.125
                )
                if qb == qb0:
                    for hp in range(2):
                        nc.gpsimd.tensor_mul(
                            et[:, hp, c0 : c0 + 128],
                            et[:, hp, c0 : c0 + 128],
                            mdiag_sb[:, :],
                        )
            else:
                et, c0 = None, None
            if pending is not None:
                pkt, pc0, pet = pending
                for hp in range(2):
                    h = 2 * m + hp
                    nc.tensor.matmul(
                        puv[:, hp, pc0:512],
                        vext[:, pkt, h * EV : h * EV + 65],
                        pet[:, hp, pc0:512],
                        start=(pkt == 0),
                        stop=(pkt == qb * 4 + 3),
                        skip_group_check=True,
                    )
            pending = (kt, c0, et) if et is not None else None
            if fillers and (kt % pace) == pace - 1:
                fillers.pop(0)()
        # evacuate puv quickly; the denominator row first (it heads the
        # critical reciprocal chain), then the vec cast (uv bufs=1)
        srow = rrow_p.tile([1, 2, 512], F32, tag="rr", name=f"srow_{m}_{qb}")
        nc.vector.tensor_copy(srow[:, :, :], puv[64:65, :, :])
        vu = vu_p.tile([64, 2, 512], BF16, tag="vu", name=f"vu_{m}_{qb}")
        nc.vector.tensor_copy(vu[:, :, :], puv[0:64, :, :])
        normalize_pair(m, qb, vu, srow)
        while fillers:
            fillers.pop(0)()

    to_tiles = {}

    def outproj_chunk(qt_i, oc):
        qsl = slice(qt_i * 128, (qt_i + 1) * 128)
        osl = slice(oc * 512, (oc + 1) * 512)
        pso = ps_pool.tile([128, 512], F32, tag="w", name="pso")
        for mt in range(MT):
            nc.tensor.matmul(
                pso[:, :],
                vecT[:, mt, qsl],
                wo_sb[:, mt, osl],
                start=(mt == 0),
                stop=(mt == MT - 1),
            )
        if oc == 0:
            to_tiles[qt_i] = tout_p.tile([128, 1024], BF16, tag="to", name=f"to_{qt_i}")
        to = to_tiles[qt_i]
        if (qt_i + oc) % 2 == 0:
            nc.scalar.copy(to[:, osl], pso[:, :])
        else:
            nc.vector.tensor_copy(to[:, osl], pso[:, :])
        if oc == 1:
            nc.sync.dma_start(out_ap[qsl, :], to[:, :])
            del to_tiles[qt_i]

    def outproj_chunks(qb):
        return [
            (lambda qt_i=qt_i, oc=oc: outproj_chunk(qt_i, oc))
            for qt_i in range(4 * qb, 4 * qb + 4)
            for oc in range(2)
        ]

    # ---- main pipeline ----
    for c in proj_chunks(0, xts[0]):
        c()
    for qb in range(NQB):
        fillers = []
        if qb + 1 < NQB:
            fillers += proj_chunks(qb + 1, xts[qb + 1])
        if qb >= 1:
            fillers += outproj_chunks(qb - 1)
        points = 2 * (4 * qb + 5)
        pace = max(1, points // max(1, len(fillers)))
        half = len(fillers) // 2 if len(fillers) > 8 else len(fillers)
        attn_qb(0, qb, fillers[:half], pace)
        attn_qb(1, qb, fillers[half:], pace)
    for c in outproj_chunks(NQB - 1):
        c()
    ctx._dbg = {"qrt": qrt, "krt": krt, "vext": vext, "vecT": vecT}


def build_bass(fix_waits=True, debug_dump=False):
    nc = bass.Bass("TRN2", debug=False)
    inp = {}

    def din(name, shape, dtype=BF16):
        inp[name] = nc.dram_tensor(name, list(shape), dtype, kind="ExternalInput").ap()

    din("xT", (D, S))
    din("wqT", (D, DC))
    din("wkT", (D, DC))
    din("wvT", (D, DC))
    din("bqc", (128, MT), F32)
    din("bvr", (1, DC))
    din("cosT", (128, S))
    din("sinT", (128, S))
    din("mdiagT", (128, 128))
    din("woT", (DC, D))
    din("ones", (1, 128))
    din("pshift", (128, 128))
    din("vones", (128, NST * HPC))
    inp["recr"] = nc.dram_tensor("recr", [MT * NQB, 1024], BF16, kind="Internal").ap()
    out_ap = nc.dram_tensor("out", [S, D], BF16, kind="ExternalOutput").ap()

    if debug_dump:
        for nm, shape in (("qrt_d", [128, MT * S]), ("krt_d", [128, MT * S]),
                          ("vext_d", [128, NST * HPC * EV]),
                          ("vecT_d", [128, MT * S])):
            inp[nm] = nc.dram_tensor(nm, shape, BF16, kind="ExternalOutput").ap()

    with tile.TileContext(nc) as tc:
        with ExitStack() as ctx:
            _attention_body(ctx, tc, inp, out_ap)
            if debug_dump:
                nc.sync.dma_start(inp["qrt_d"], ctx._dbg["qrt"].rearrange("p a b -> p (a b)"))
                nc.sync.dma_start(inp["krt_d"], ctx._dbg["krt"].rearrange("p a b -> p (a b)"))
                nc.sync.dma_start(inp["vext_d"], ctx._dbg["vext"].rearrange("p a b -> p (a b)"))
                nc.sync.dma_start(inp["vecT_d"], ctx._dbg["vecT"].rearrange("p a b -> p (a b)"))
    if fix_waits:
        fix_engine_waits(nc)
    return nc


# ---- host-side sharding / prep ----


def make_core_inputs(x, mask, cos, sin, wq, bq, wk, wv, bv, wo):
    """Returns list of 8 input dicts (core c = batch c//4, head-group c%4)."""
    x = np.ascontiguousarray(x, dtype=np.float32)
    p = np.arange(128)
    pf = p % 64
    cosT = np.ascontiguousarray(cos.T[pf, :].astype(BF))                 # [128, S]
    sinT = np.ascontiguousarray(sin.T[pf, :].astype(BF))
    # rotate_half as a permutation matmul: psh[p,s] = sgn(p) * qt[p^32, s]
    # pshift is the stationary lhsT: pshift[q, p] = sgn(p) * (q == p ^ 32)
    sgn = np.where(pf < 32, -1.0, 1.0).astype(np.float32)
    pshift = np.zeros((128, 128), dtype=np.float32)
    pshift[p ^ 32, p] = sgn
    pshift = np.ascontiguousarray(pshift.astype(BF))
    mdiagT = np.ascontiguousarray(
        (mask[0:128, 0:128].T == 0).astype(BF)
    )
    ones = np.ones((1, 128), dtype=BF)
    vones = np.ones((128, NST * HPC), dtype=BF)

    in_maps = []
    for c in range(8):
        b, g = c // 4, c % 4
        rows = np.arange(g * DC, (g + 1) * DC)
        vrows = rows
        bqc = np.ascontiguousarray(bq[rows].reshape(MT, 128).T, dtype=np.float32)
        in_maps.append({
            "xT": np.ascontiguousarray(x[b].T.astype(BF)),
            "wqT": np.ascontiguousarray(wq[rows].T.astype(BF)),
            "wkT": np.ascontiguousarray(wk[rows].T.astype(BF)),
            "wvT": np.ascontiguousarray(wv[vrows].T.astype(BF)),
            "bqc": bqc,
            "bvr": np.ascontiguousarray(bv[vrows][None, :].astype(BF)),
            "cosT": cosT,
            "sinT": sinT,
            "mdiagT": mdiagT,
            "woT": np.ascontiguousarray(wo[:, vrows].T.astype(BF)),
            "ones": ones,
            "pshift": pshift,
            "vones": vones,
        })
    return in_maps


_NC_CACHE = []


def kernel(x, mask, cos, sin, wq, bq, wk, wv, bv, wo, bo):
    x = np.asarray(x, dtype=np.float32)
    in_maps = make_core_inputs(
        x, np.asarray(mask), np.asarray(cos), np.asarray(sin),
        np.asarray(wq), np.asarray(bq), np.asarray(wk), np.asarray(wv),
        np.asarray(bv), np.asarray(wo),
    )
    if not _NC_CACHE:
        _NC_CACHE.append(build_bass())
    nc = _NC_CACHE[0]
    res = run_bass_kernel_spmd(nc, in_maps, core_ids=list(range(8)))
    out = np.zeros((B, S, D), dtype=np.float32)
    for c in range(8):
        out[c // 4] += res.results[c]["out"].astype(np.float32)
    out += np.asarray(bo, dtype=np.float32)[None, None, :]
    return out
